# revision 1
# baseline (speedup 1.0000x reference)
"""DTW similarity kernel for Trainium2 (8 NeuronCores, SPMD bass/tile).

Per core (replicated; inputs identical on all cores):
  1. L2-normalize trajectory rows -> bf16 (DRAM bounce).
  2. DMA-transpose loads -> [D, N] bf16 operands in SBUF.
  3. cost = 1 - t1n @ t2n.T via PE matmuls -> f32 cost matrix in DRAM.
  4. DTW DP, skewed wavefront: strips of 128 rows on partitions, block
     width C. Cross-partition "up row" comes from a PE shift-matrix
     matmul into PSUM (SH1: out[p]=in[p-1]; SH2 injects the previous
     strip's last row into partition 0). ScalarE copies PSUM->SBUF,
     then VectorE: a = min(up, updiag); tensor_tensor_scan computes
     D_j = c_j + min(a_j, D_{j-1}) along the block.
  5. similarity = 1/(1+distance) -> scalar output.
"""

import sys

sys.path.insert(0, "/opt/trn_rl_repo")

import numpy as np  # noqa: E402

BIG = 1e30
NCORES = 8


def _build(N1, N2, D, C):
    from concourse import bacc
    import concourse.bass as bass
    import concourse.mybir as mybir
    import concourse.tile as tile

    f32 = mybir.dt.float32
    bf16 = mybir.dt.bfloat16
    P = 128
    assert N1 % P == 0 and N2 % C == 0 and D % P == 0 and N2 % 512 == 0
    nstrips = N1 // P
    B = N2 // C
    W = N2 + (P - 1) * C + 1
    KT = D // P
    NT = N2 // 512
    MT = N1 // P

    nc = bacc.Bacc(None, target_bir_lowering=False, debug=True, num_devices=NCORES)
    t1_ext = nc.dram_tensor("trajectory1", [N1, D], f32, kind="ExternalInput")
    t2_ext = nc.dram_tensor("trajectory2", [N2, D], f32, kind="ExternalInput")
    out_ext = nc.dram_tensor("out", [1, 1], f32, kind="ExternalOutput")

    mn = mybir.AluOpType.min
    ad = mybir.AluOpType.add
    ml = mybir.AluOpType.mult
    eq = mybir.AluOpType.is_equal
    AF = mybir.ActivationFunctionType

    with tile.TileContext(nc) as tc:
        with tc.tile_pool(name="dram", bufs=1, space="DRAM") as dram:
            t1n_dram = dram.tile([N1, D], bf16)
            t2n_dram = dram.tile([N2, D], bf16)
            cost_dram = dram.tile([N1, N2], f32)

            # ---- phase A: normalize rows, write bf16 bounce ----
            with tc.tile_pool(name="norm", bufs=3) as pn, \
                 tc.tile_pool(name="norms", bufs=4) as ps:
                for src, dst, n in ((t1_ext, t1n_dram, N1), (t2_ext, t2n_dram, N2)):
                    for i in range(n // P):
                        x = pn.tile([P, D], f32, tag="x")
                        nc.sync.dma_start(out=x[:], in_=src[i * P:(i + 1) * P, :])
                        sq = pn.tile([P, D], f32, tag="sq")
                        ss = ps.tile([P, 1], f32, tag="ss")
                        nc.scalar.activation(
                            out=sq[:], in_=x[:], func=AF.Square, accum_out=ss[:])
                        nc.scalar.activation(out=ss[:], in_=ss[:], func=AF.Sqrt)
                        nc.vector.tensor_scalar(ss[:], ss[:], 1e-8, None, ad)
                        r = ps.tile([P, 1], f32, tag="r")
                        nc.vector.reciprocal(r[:], ss[:])
                        y = pn.tile([P, D], bf16, tag="y")
                        nc.vector.tensor_tensor(
                            y[:], x[:], r[:].to_broadcast((P, D)), ml)
                        nc.sync.dma_start(out=dst[i * P:(i + 1) * P, :], in_=y[:])

            # ---- phase B+C: transposed loads + GEMM ----
            with tc.tile_pool(name="ops", bufs=1) as pg, \
                 tc.tile_pool(name="psum", bufs=4, space="PSUM") as pp, \
                 tc.tile_pool(name="bounce", bufs=4) as pb:
                t1T = []
                t2T = []
                for k in range(KT):
                    a = pg.tile([P, N1], bf16, tag=f"t1T{k}")
                    nc.sync.dma_start_transpose(a[:], t1n_dram[:, k * P:(k + 1) * P])
                    t1T.append(a)
                for k in range(KT):
                    a = pg.tile([P, N2], bf16, tag=f"t2T{k}")
                    nc.sync.dma_start_transpose(a[:], t2n_dram[:, k * P:(k + 1) * P])
                    t2T.append(a)
                for m in range(MT):
                    for n in range(NT):
                        acc = pp.tile([P, 512], f32, tag="acc")
                        for k in range(KT):
                            nc.tensor.matmul(
                                acc[:],
                                t1T[k][:, m * P:(m + 1) * P],
                                t2T[k][:, n * 512:(n + 1) * 512],
                                start=(k == 0), stop=(k == KT - 1))
                        b = pb.tile([P, 512], f32, tag="b")
                        nc.vector.tensor_scalar(b[:], acc[:], -1.0, 1.0, ml, ad)
                        nc.sync.dma_start(
                            out=cost_dram[m * P:(m + 1) * P, n * 512:(n + 1) * 512],
                            in_=b[:])

            # ---- phase D: DTW DP ----
            with tc.tile_pool(name="dconst", bufs=1) as pc, \
                 tc.tile_pool(name="dcs", bufs=2) as pcs, \
                 tc.tile_pool(name="dD", bufs=2) as pD, \
                 tc.tile_pool(name="dps", bufs=6, space="PSUM") as pps, \
                 tc.tile_pool(name="da", bufs=8) as pa:
                # constants: shift matrices, BIG tile, iotas
                iop = pc.tile([P, P], f32, tag="iop")   # value = partition idx
                iof = pc.tile([P, P], f32, tag="iof")   # value = free idx
                nc.gpsimd.iota(iof[:], [[1, P]], channel_multiplier=0, allow_small_or_imprecise_dtypes=True)
                nc.gpsimd.iota(iop[:], [[0, P]], channel_multiplier=1, allow_small_or_imprecise_dtypes=True)
                sh1 = pc.tile([P, P], f32, tag="sh1")   # sh1[k,m] = (k+1 == m)
                nc.vector.tensor_scalar(sh1[:], iop[:], 1.0, None, ad)
                nc.vector.tensor_tensor(sh1[:], sh1[:], iof[:], eq)
                sh2 = pc.tile([P, P], f32, tag="sh2")   # sh2[k,m] = (k==127)*(m==0)
                tmp = pc.tile([P, P], f32, tag="tmp")
                nc.vector.tensor_scalar(sh2[:], iop[:], float(P - 1), None, eq)
                nc.vector.tensor_scalar(tmp[:], iof[:], 0.0, None, eq)
                nc.vector.tensor_tensor(sh2[:], sh2[:], tmp[:], ml)
                bigt = pc.tile([P, C + 1], f32, tag="bigt")
                nc.vector.memset(bigt[:], BIG)

                cost_base = cost_dram[:, :]
                Dprev = None
                for s in range(nstrips):
                    cs = pcs.tile([P, W], f32, tag="cs")
                    src = bass.AP(
                        tensor=cost_base.tensor,
                        offset=cost_base.offset + s * P * N2,
                        ap=[[N2 - C, P], [1, W - 1]])
                    nc.sync.dma_start(out=cs[:, 1:W], in_=src)
                    Dt = pD.tile([P, W], f32, tag="Dt")
                    nc.gpsimd.memset(Dt[:], BIG)
                    for t in range(B + P - 1):
                        phi = min(t, P - 1)
                        na = phi + 1
                        tm = min(t, B - 1)  # clamp for in-bounds junk reads
                        ps_t = pps.tile([P, C + 1], f32, tag="shps")
                        if t > 0:
                            nc.tensor.matmul(
                                ps_t[0:na, :], sh1[:, 0:na],
                                Dt[:, (t - 1) * C:t * C + 1],
                                start=True, stop=False)
                        rhs2 = (bigt[:, 0:C + 1] if s == 0 else
                                Dprev[:, (tm + P - 1) * C:(tm + P - 1) * C + C + 1])
                        nc.tensor.matmul(
                            ps_t[0:na, :], sh2[:, 0:na], rhs2,
                            start=(t == 0), stop=True)
                        u = pa.tile([P, C + 1], f32, tag="u")
                        nc.scalar.activation(
                            out=u[0:na, :], in_=ps_t[0:na, :], func=AF.Copy)
                        a = pa.tile([P, C], f32, tag="a")
                        nc.vector.tensor_tensor(
                            a[0:na, :], u[0:na, 1:C + 1], u[0:na, 0:C], mn)
                        if s == 0 and t == 0:
                            nc.vector.memset(a[0:1, 0:1], 0.0)
                        xs = t * C + 1
                        nc.vector.tensor_tensor_scan(
                            Dt[0:na, xs:xs + C],
                            a[0:na, :],
                            cs[0:na, xs:xs + C],
                            Dt[0:na, xs - 1:xs],
                            mn, ad)
                    Dprev = Dt

                # similarity = 1/(1+distance); move off partition 127 via DMA
                r0 = pa.tile([1, 1], f32, tag="r0")
                nc.sync.dma_start(out=r0[:], in_=Dprev[P - 1:P, W - 1:W])
                nc.vector.tensor_scalar(r0[:], r0[:], 1.0, None, ad)
                nc.vector.reciprocal(r0[:], r0[:])
                nc.sync.dma_start(out=out_ext[:], in_=r0[:])

    nc.finalize()
    return nc


_cache = {}


def _get_nc(N1, N2, D, C):
    key = (N1, N2, D, C)
    if key not in _cache:
        _cache[key] = _build(N1, N2, D, C)
    return _cache[key]


def run(trajectory1, trajectory2, C=64, trace=False):
    from concourse.bass_utils import run_bass_kernel_spmd

    N1, D = trajectory1.shape
    N2, _ = trajectory2.shape
    nc = _get_nc(N1, N2, D, C)
    in_map = {
        "trajectory1": np.ascontiguousarray(trajectory1, dtype=np.float32),
        "trajectory2": np.ascontiguousarray(trajectory2, dtype=np.float32),
    }
    res = run_bass_kernel_spmd(
        nc, [in_map] * NCORES, list(range(NCORES)), trace=trace)
    out = res.results[0]["out"]
    return np.float32(out.reshape(())), res


def kernel(trajectory1, trajectory2):
    out, _ = run(trajectory1, trajectory2)
    return out



# revision 5
# speedup vs baseline: 2.3444x; 2.3444x over previous
"""DTW similarity kernel for Trainium2 (8 NeuronCores, SPMD bass/tile).

Per core (replicated; inputs identical on all cores):
  1. L2-normalize trajectory rows -> bf16 (DRAM bounce).
  2. DMA-transpose loads -> [D, N] bf16 operands in SBUF.
  3. cost = 1 - t1n @ t2n.T via PE matmuls -> bf16 cost matrix in DRAM.
  4. DTW DP over the NEGATED value space (D' = -D so min() becomes max(),
     letting max-pool do the up/diag candidate merge):
       - strips of 256 rows: partition p owns rows {2p, 2p+1}; skewed
         block wavefront, block width C, 2 rows chained per step.
       - cross-partition "up" for even rows via a PE shift matmul into
         PSUM (sh1: out[p]=in[p-1]; sh2 injects prev strip's last row).
       - aA = window-2 max-pool straight off PSUM; scanA computes the
         even row (tensor_tensor_scan, op0=max, op1=subtract).
       - odd row reads the fresh even row in SBUF (same partition):
         TT-max + scanB.
       - strips are emitted interleaved along the global wavefront so
         two strips pipeline across engines.
  5. similarity = 1/(1+distance) -> scalar output.
"""

import sys

sys.path.insert(0, "/opt/trn_rl_repo")

import numpy as np  # noqa: E402

BIG = 1e30
NCORES = 8


def _build(N1, N2, D, C):
    from concourse import bacc
    import concourse.bass as bass
    import concourse.mybir as mybir
    import concourse.tile as tile

    f32 = mybir.dt.float32
    bf16 = mybir.dt.bfloat16
    P = 128
    R = 2                      # rows per partition
    ROWS = P * R               # rows per strip
    S = N1 // ROWS             # strips
    B = N2 // C                # blocks per row
    NSTEP = B + P - 1          # wavefront steps per strip
    W2 = NSTEP * C + 1         # skewed D/cs tile width
    RING = 4                   # even-row D ring depth (blocks)
    RW = RING * C + 1
    STAG = P                   # wavefront stagger between strips
    KT = D // P
    NT = N2 // 512
    MT = N1 // P
    assert N1 % ROWS == 0 and N2 % C == 0 and D % P == 0 and N2 % 512 == 0

    nc = bacc.Bacc(None, target_bir_lowering=False, debug=True, num_devices=NCORES)
    t1_ext = nc.dram_tensor("trajectory1", [N1, D], f32, kind="ExternalInput")
    t2_ext = nc.dram_tensor("trajectory2", [N2, D], f32, kind="ExternalInput")
    out_ext = nc.dram_tensor("out", [1, 1], f32, kind="ExternalOutput")

    ad = mybir.AluOpType.add
    ml = mybir.AluOpType.mult
    mx = mybir.AluOpType.max
    sb = mybir.AluOpType.subtract
    eq = mybir.AluOpType.is_equal
    AF = mybir.ActivationFunctionType
    PMAX = mybir.PoolFunctionType.max

    with tile.TileContext(nc) as tc:
        with tc.tile_pool(name="dram", bufs=1, space="DRAM") as dram:
            t1n_dram = dram.tile([N1, D], bf16)
            t2n_dram = dram.tile([N2, D], bf16)
            cost_dram = dram.tile([N1, N2], bf16)

            # ---- phase A: normalize rows, write bf16 bounce ----
            with tc.tile_pool(name="norm", bufs=3) as pn, \
                 tc.tile_pool(name="norms", bufs=4) as psn:
                for src, dst, n in ((t1_ext, t1n_dram, N1), (t2_ext, t2n_dram, N2)):
                    for i in range(n // P):
                        x = pn.tile([P, D], f32, tag="x")
                        nc.sync.dma_start(out=x[:], in_=src[i * P:(i + 1) * P, :])
                        sq = pn.tile([P, D], f32, tag="sq")
                        ss = psn.tile([P, 1], f32, tag="ss")
                        nc.scalar.activation(
                            out=sq[:], in_=x[:], func=AF.Square, accum_out=ss[:])
                        nc.scalar.activation(out=ss[:], in_=ss[:], func=AF.Sqrt)
                        nc.vector.tensor_scalar(ss[:], ss[:], 1e-8, None, ad)
                        r = psn.tile([P, 1], f32, tag="r")
                        nc.vector.reciprocal(r[:], ss[:])
                        y = pn.tile([P, D], bf16, tag="y")
                        nc.vector.tensor_tensor(
                            y[:], x[:], r[:].to_broadcast((P, D)), ml)
                        nc.sync.dma_start(out=dst[i * P:(i + 1) * P, :], in_=y[:])

            # ---- phase B+C: transposed loads + GEMM (cost in bf16) ----
            with tc.tile_pool(name="ops", bufs=1) as pg, \
                 tc.tile_pool(name="psum", bufs=4, space="PSUM") as pp, \
                 tc.tile_pool(name="bounce", bufs=4) as pb:
                t1T = []
                t2T = []
                for k in range(KT):
                    a = pg.tile([P, N1], bf16, tag=f"t1T{k}")
                    nc.sync.dma_start_transpose(a[:], t1n_dram[:, k * P:(k + 1) * P])
                    t1T.append(a)
                for k in range(KT):
                    a = pg.tile([P, N2], bf16, tag=f"t2T{k}")
                    nc.sync.dma_start_transpose(a[:], t2n_dram[:, k * P:(k + 1) * P])
                    t2T.append(a)
                for m in range(MT):
                    for n in range(NT):
                        acc = pp.tile([P, 512], f32, tag="acc")
                        for k in range(KT):
                            nc.tensor.matmul(
                                acc[:],
                                t1T[k][:, m * P:(m + 1) * P],
                                t2T[k][:, n * 512:(n + 1) * 512],
                                start=(k == 0), stop=(k == KT - 1))
                        b = pb.tile([P, 512], bf16, tag="b")
                        nc.vector.tensor_scalar(b[:], acc[:], -1.0, 1.0, ml, ad)
                        nc.sync.dma_start(
                            out=cost_dram[m * P:(m + 1) * P, n * 512:(n + 1) * 512],
                            in_=b[:])

            # ---- phase D: DTW DP (negated), interleaved wavefront ----
            with tc.tile_pool(name="dconst", bufs=1) as pc, \
                 tc.tile_pool(name="dcs", bufs=3) as pcs, \
                 tc.tile_pool(name="dDB", bufs=3) as pDB, \
                 tc.tile_pool(name="dDA", bufs=3) as pDA, \
                 tc.tile_pool(name="dps", bufs=8, space="PSUM") as pps, \
                 tc.tile_pool(name="da", bufs=8) as pa:
                # constants: shift matrices, -BIG tile, iotas
                iop = pc.tile([P, P], f32, tag="iop")   # value = partition idx
                iof = pc.tile([P, P], f32, tag="iof")   # value = free idx
                nc.gpsimd.iota(iof[:], [[1, P]], channel_multiplier=0,
                               allow_small_or_imprecise_dtypes=True)
                nc.gpsimd.iota(iop[:], [[0, P]], channel_multiplier=1,
                               allow_small_or_imprecise_dtypes=True)
                sh1 = pc.tile([P, P], f32, tag="sh1")   # sh1[k,m] = (k+1 == m)
                nc.vector.tensor_scalar(sh1[:], iop[:], 1.0, None, ad)
                nc.vector.tensor_tensor(sh1[:], sh1[:], iof[:], eq)
                sh2 = pc.tile([P, P], f32, tag="sh2")   # sh2[k,m] = (k==127)*(m==0)
                tmp = pc.tile([P, P], f32, tag="tmp")
                nc.vector.tensor_scalar(sh2[:], iop[:], float(P - 1), None, eq)
                nc.vector.tensor_scalar(tmp[:], iof[:], 0.0, None, eq)
                nc.vector.tensor_tensor(sh2[:], sh2[:], tmp[:], ml)
                bigt = pc.tile([P, C + 1], f32, tag="bigt")
                nc.vector.memset(bigt[:], -BIG)

                csA = {}
                csB = {}
                DtA = {}
                DtB = {}

                def setup_strip(s):
                    # skewed bf16 cost loads: partition p <- rows 256s+2p / +1
                    for tag, par, dst in (("csA", 0, csA), ("csB", 1, csB)):
                        cs = pcs.tile([P, W2], bf16, tag=tag)
                        src = bass.AP(
                            tensor=cost_dram[:, :].tensor,
                            offset=cost_dram[:, :].offset
                            + (s * ROWS + par) * N2,
                            ap=[[2 * N2 - C, P], [1, W2 - 1]])
                        nc.sync.dma_start(out=cs[:, 1:W2], in_=src)
                        dst[s] = cs
                    db = pDB.tile([P, W2], f32, tag="DtB")
                    nc.gpsimd.memset(db[:], -BIG)
                    DtB[s] = db
                    da = pDA.tile([P, RW], f32, tag="DtA")
                    nc.gpsimd.memset(da[:], -BIG)
                    DtA[s] = da

                def emit_step(s, t):
                    na = min(t, P - 1) + 1
                    tm = min(t, B - 1)
                    xs = t * C + 1
                    j = t % RING
                    ps = pps.tile([P, C + 1], f32, tag="ps")
                    # carry (prev strip's last row -> partition 0)
                    rhs2 = (bigt[:, 0:C + 1] if s == 0 else
                            DtB[s - 1][:, (tm + P - 1) * C:(tm + P - 1) * C + C + 1])
                    nc.tensor.matmul(
                        ps[0:na, :], sh2[:, 0:na], rhs2,
                        start=True, stop=(t == 0))
                    if t > 0:
                        nc.tensor.matmul(
                            ps[0:na, :], sh1[:, 0:na],
                            DtB[s][:, (t - 1) * C:t * C + 1],
                            start=False, stop=True)
                    # even row: copy shifted window off PSUM, then window-2 max
                    u = pa.tile([P, C + 1], f32, tag="u")
                    nc.scalar.activation(
                        out=u[0:na, :], in_=ps[0:na, :], func=AF.Copy)
                    aA = pa.tile([P, C], f32, tag="aA")
                    nc.vector.tensor_tensor(
                        aA[0:na, :], u[0:na, 1:C + 1], u[0:na, 0:C], mx)
                    if s == 0 and t == 0:
                        nc.vector.memset(aA[0:1, 0:1], 0.0)
                    dA = DtA[s]
                    pred = j * C if j > 0 else RING * C
                    nc.vector.tensor_tensor_scan(
                        dA[0:na, j * C + 1:j * C + C + 1],
                        aA[0:na, :],
                        csA[s][0:na, xs:xs + C],
                        dA[0:na, pred:pred + 1],
                        mx, sb)
                    # odd row: candidates from the fresh even row (same partition)
                    aB = pa.tile([P, C], f32, tag="aB")
                    if j > 0:
                        nc.vector.tensor_tensor(
                            aB[0:na, :],
                            dA[0:na, j * C + 1:j * C + C + 1],
                            dA[0:na, j * C:j * C + C], mx)
                    else:
                        nc.vector.tensor_tensor(
                            aB[0:na, 0:1], dA[0:na, 1:2],
                            dA[0:na, RING * C:RING * C + 1], mx)
                        nc.vector.tensor_tensor(
                            aB[0:na, 1:C], dA[0:na, 2:C + 1],
                            dA[0:na, 1:C], mx)
                    nc.vector.tensor_tensor_scan(
                        DtB[s][0:na, xs:xs + C],
                        aB[0:na, :],
                        csB[s][0:na, xs:xs + C],
                        DtB[s][0:na, xs - 1:xs],
                        mx, sb)

                setup_strip(0)
                TOT = STAG * (S - 1) + NSTEP
                for T in range(TOT):
                    if T % STAG == 64 and T // STAG + 1 < S:
                        setup_strip(T // STAG + 1)
                    s_hi = min(T // STAG, S - 1)
                    s_lo = max(0, -(-(T - (NSTEP - 1)) // STAG))
                    for s in range(s_lo, s_hi + 1):
                        t = T - STAG * s
                        if 0 <= t < NSTEP:
                            emit_step(s, t)

                # similarity = 1/(1+distance); D' = -distance on partition 127
                r0 = pa.tile([1, 1], f32, tag="r0")
                nc.sync.dma_start(out=r0[:], in_=DtB[S - 1][P - 1:P, W2 - 1:W2])
                nc.vector.tensor_scalar(r0[:], r0[:], -1.0, 1.0, ml, ad)
                nc.vector.reciprocal(r0[:], r0[:])
                nc.sync.dma_start(out=out_ext[:], in_=r0[:])

    nc.finalize()
    return nc


_cache = {}


def _get_nc(N1, N2, D, C):
    key = (N1, N2, D, C)
    if key not in _cache:
        _cache[key] = _build(N1, N2, D, C)
    return _cache[key]


def run(trajectory1, trajectory2, C=32, trace=False):
    from concourse.bass_utils import run_bass_kernel_spmd

    N1, D = trajectory1.shape
    N2, _ = trajectory2.shape
    nc = _get_nc(N1, N2, D, C)
    in_map = {
        "trajectory1": np.ascontiguousarray(trajectory1, dtype=np.float32),
        "trajectory2": np.ascontiguousarray(trajectory2, dtype=np.float32),
    }
    res = run_bass_kernel_spmd(
        nc, [in_map] * NCORES, list(range(NCORES)), trace=trace)
    out = res.results[0]["out"]
    return np.float32(out.reshape(())), res


def kernel(trajectory1, trajectory2):
    out, _ = run(trajectory1, trajectory2)
    return out


# revision 21
# speedup vs baseline: 2.5751x; 1.0984x over previous
"""DTW similarity kernel for Trainium2 (8 NeuronCores, SPMD bass/tile).

Per core (replicated; inputs identical on all cores):
  1. L2-normalize trajectory rows -> bf16 (DRAM bounce).
  2. DMA-transpose loads -> [D, N] bf16 operands in SBUF.
  3. cost = 1 - t1n @ t2n.T via PE matmuls -> bf16 cost matrix in DRAM.
  4. DTW DP over the NEGATED value space (D' = -D so min() becomes max(),
     letting max-pool do the up/diag candidate merge):
       - strips of 256 rows: partition p owns rows {2p, 2p+1}; skewed
         block wavefront, block width C, 2 rows chained per step.
       - cross-partition "up" for even rows via a PE shift matmul into
         PSUM (sh1: out[p]=in[p-1]; sh2 injects prev strip's last row).
       - aA = window-2 max-pool straight off PSUM; scanA computes the
         even row (tensor_tensor_scan, op0=max, op1=subtract).
       - odd row reads the fresh even row in SBUF (same partition):
         TT-max + scanB.
       - strips are emitted interleaved along the global wavefront so
         two strips pipeline across engines.
  5. similarity = 1/(1+distance) -> scalar output.
"""

import sys

sys.path.insert(0, "/opt/trn_rl_repo")

import numpy as np  # noqa: E402

BIG = 1e30
NCORES = 8


def _build(N1, N2, D, C):
    from concourse import bacc
    import concourse.bass as bass
    import concourse.mybir as mybir
    import concourse.tile as tile

    f32 = mybir.dt.float32
    bf16 = mybir.dt.bfloat16
    P = 128
    R = 2                      # rows per partition
    ROWS = P * R               # rows per strip
    S = N1 // ROWS             # strips
    B = N2 // C                # blocks per row
    NSTEP = B + P - 1          # wavefront steps per strip
    W2 = NSTEP * C + 1         # skewed D/cs tile width
    RING = 16                  # even-row D ring depth (blocks)
    RW = RING * C + 1
    STAG = P                   # wavefront stagger between strips
    KT = D // P
    NT = N2 // 512
    MT = N1 // P
    assert N1 % ROWS == 0 and N2 % C == 0 and D % P == 0 and N2 % 512 == 0

    MY = MT // NCORES          # row-blocks of t1 computed per core
    NROW = MY * P              # t1 rows used per core (host permutes them first)

    nc = bacc.Bacc(None, target_bir_lowering=False, debug=True, num_devices=NCORES)
    t1_ext = nc.dram_tensor("trajectory1", [N1, D], f32, kind="ExternalInput")
    t2_ext = nc.dram_tensor("trajectory2", [N2, D], f32, kind="ExternalInput")
    out_ext = nc.dram_tensor("out", [1, 1], f32, kind="ExternalOutput")
    # GEMM shard: AllGather #g over ranks assembles slab row-block g of every
    # core into agout rows [1024g, 1024(g+1)) = original rows (host
    # permutation is strip-cyclic: core r's g-th block is orig block 8g+r).

    ad = mybir.AluOpType.add
    ml = mybir.AluOpType.mult
    mx = mybir.AluOpType.max
    sb = mybir.AluOpType.subtract
    eq = mybir.AluOpType.is_equal
    AF = mybir.ActivationFunctionType
    PMAX = mybir.PoolFunctionType.max

    with tile.TileContext(nc) as tc:
        with tc.tile_pool(name="dram", bufs=1, space="DRAM") as dram:
            t1n_dram = dram.tile([NROW, D], bf16)
            t2n_dram = dram.tile([N2, D], bf16)
            slab = dram.tile([NROW, N2], bf16)
            agout = [dram.tile([NCORES * P, N2], bf16, addr_space="Shared",
                               name=f"agout{g}")
                     for g in range(MY)]

            # ---- phase A: normalize rows, write bf16 bounce ----
            with tc.tile_pool(name="norm", bufs=3) as pn, \
                 tc.tile_pool(name="norms", bufs=4) as psn:
                for src, dst, n in ((t1_ext, t1n_dram, NROW), (t2_ext, t2n_dram, N2)):
                    for i in range(n // P):
                        x = pn.tile([P, D], f32, tag="x")
                        nc.sync.dma_start(out=x[:], in_=src[i * P:(i + 1) * P, :])
                        sq = pn.tile([P, D], f32, tag="sq")
                        ss = psn.tile([P, 1], f32, tag="ss")
                        nc.scalar.activation(
                            out=sq[:], in_=x[:], func=AF.Square, accum_out=ss[:])
                        nc.scalar.activation(out=ss[:], in_=ss[:], func=AF.Sqrt)
                        nc.vector.tensor_scalar(ss[:], ss[:], 1e-8, None, ad)
                        r = psn.tile([P, 1], f32, tag="r")
                        nc.vector.reciprocal(r[:], ss[:])
                        y = pn.tile([P, D], bf16, tag="y")
                        nc.vector.tensor_tensor(
                            y[:], x[:], r[:].to_broadcast((P, D)), ml)
                        nc.sync.dma_start(out=dst[i * P:(i + 1) * P, :], in_=y[:])

            # ---- phase B+C: transposed loads + GEMM (cost in bf16) ----
            with tc.tile_pool(name="ops", bufs=1) as pg, \
                 tc.tile_pool(name="psum", bufs=4, space="PSUM") as pp, \
                 tc.tile_pool(name="bounce", bufs=4) as pb:
                t1T = []
                t2T = []
                for k in range(KT):
                    a = pg.tile([P, NROW], bf16, tag=f"t1T{k}")
                    nc.sync.dma_start_transpose(a[:], t1n_dram[:, k * P:(k + 1) * P])
                    t1T.append(a)
                for k in range(KT):
                    a = pg.tile([P, N2], bf16, tag=f"t2T{k}")
                    nc.sync.dma_start_transpose(a[:], t2n_dram[:, k * P:(k + 1) * P])
                    t2T.append(a)
                for m in range(MY):
                    for n in range(NT):
                        acc = pp.tile([P, 512], f32, tag="acc")
                        for k in range(KT):
                            nc.tensor.matmul(
                                acc[:],
                                t1T[k][:, m * P:(m + 1) * P],
                                t2T[k][:, n * 512:(n + 1) * 512],
                                start=(k == 0), stop=(k == KT - 1))
                        b = pb.tile([P, 512], bf16, tag="b")
                        nc.vector.tensor_scalar(b[:], acc[:], -1.0, 1.0, ml, ad)
                        nc.sync.dma_start(
                            out=slab[m * P:(m + 1) * P, n * 512:(n + 1) * 512],
                            in_=b[:])
                # pipelined all-gathers: group g = slab block g of every core
                for g in range(MY):
                    nc.gpsimd.collective_compute(
                        "AllGather",
                        mybir.AluOpType.bypass,
                        replica_groups=[list(range(NCORES))],
                        ins=[slab[g * P:(g + 1) * P, :].opt()],
                        outs=[agout[g][:, :].opt()],
                    )

            # ---- phase D: DTW DP (negated), interleaved wavefront ----
            with tc.tile_pool(name="dconst", bufs=1) as pc, \
                 tc.tile_pool(name="dcs", bufs=3) as pcs, \
                 tc.tile_pool(name="dDB", bufs=3) as pDB, \
                 tc.tile_pool(name="dDA", bufs=3) as pDA, \
                 tc.tile_pool(name="dps", bufs=8, space="PSUM") as pps, \
                 tc.tile_pool(name="da", bufs=8) as pa:
                # constants: shift matrices, -BIG tile, iotas
                iop = pc.tile([P, P], f32, tag="iop")   # value = partition idx
                iof = pc.tile([P, P], f32, tag="iof")   # value = free idx
                nc.gpsimd.iota(iof[:], [[1, P]], channel_multiplier=0,
                               allow_small_or_imprecise_dtypes=True)
                nc.gpsimd.iota(iop[:], [[0, P]], channel_multiplier=1,
                               allow_small_or_imprecise_dtypes=True)
                sh1 = pc.tile([P, P], f32, tag="sh1")   # sh1[k,m] = (k+1 == m)
                nc.vector.tensor_scalar(sh1[:], iop[:], 1.0, None, ad)
                nc.vector.tensor_tensor(sh1[:], sh1[:], iof[:], eq)
                sh2 = pc.tile([P, P], f32, tag="sh2")   # sh2[k,m] = (k==127)*(m==0)
                tmp = pc.tile([P, P], f32, tag="tmp")
                nc.vector.tensor_scalar(sh2[:], iop[:], float(P - 1), None, eq)
                nc.vector.tensor_scalar(tmp[:], iof[:], 0.0, None, eq)
                nc.vector.tensor_tensor(sh2[:], sh2[:], tmp[:], ml)
                bigt = pc.tile([P, C + 1], f32, tag="bigt")
                nc.vector.memset(bigt[:], -BIG)

                csA = {}
                csB = {}
                DtA = {}
                DtB = {}

                def setup_strip(s):
                    # skewed bf16 cost loads: partition p <- rows 256s+2p / +1
                    g, srel = divmod(s * ROWS, NCORES * P)
                    for tag, par, dst in (("csA", 0, csA), ("csB", 1, csB)):
                        cs = pcs.tile([P, W2], bf16, tag=tag)
                        base = agout[g][:, :]
                        src = bass.AP(
                            tensor=base.tensor,
                            offset=base.offset + (srel + par) * N2,
                            ap=[[2 * N2 - C, P], [1, W2 - 1]])
                        nc.sync.dma_start(out=cs[:, 1:W2], in_=src)
                        dst[s] = cs
                    db = pDB.tile([P, W2], f32, tag="DtB")
                    nc.gpsimd.memset(db[:], -BIG)
                    DtB[s] = db
                    da = pDA.tile([P, RW], f32, tag="DtA")
                    nc.gpsimd.memset(da[:], -BIG)
                    DtA[s] = da

                def emit_step(s, t):
                    na = min(t, P - 1) + 1
                    tm = min(t, B - 1)
                    xs = t * C + 1
                    j = t % RING
                    ps = pps.tile([P, C + 1], f32, tag="ps")
                    # carry (prev strip's last row -> partition 0)
                    rhs2 = (bigt[:, 0:C + 1] if s == 0 else
                            DtB[s - 1][:, (tm + P - 1) * C:(tm + P - 1) * C + C + 1])
                    nc.tensor.matmul(
                        ps[0:na, :], sh2[:, 0:na], rhs2,
                        start=True, stop=(t == 0))
                    if t > 0:
                        nc.tensor.matmul(
                            ps[0:na, :], sh1[:, 0:na],
                            DtB[s][:, (t - 1) * C:t * C + 1],
                            start=False, stop=True)
                    # even row: copy shifted window off PSUM, then window-2 max
                    u = pa.tile([P, C + 1], f32, tag="u")
                    nc.scalar.activation(
                        out=u[0:na, :], in_=ps[0:na, :], func=AF.Copy)
                    aA = pa.tile([P, C], f32, tag="aA")
                    nc.vector.tensor_tensor(
                        aA[0:na, :], u[0:na, 1:C + 1], u[0:na, 0:C], mx)
                    if s == 0 and t == 0:
                        nc.vector.memset(aA[0:1, 0:1], 0.0)
                    dA = DtA[s]
                    pred = j * C if j > 0 else RING * C
                    nc.vector.tensor_tensor_scan(
                        dA[0:na, j * C + 1:j * C + C + 1],
                        aA[0:na, :],
                        csA[s][0:na, xs:xs + C],
                        dA[0:na, pred:pred + 1],
                        mx, sb)
                    # odd row: candidates from the fresh even row (same partition)
                    aB = pa.tile([P, C], f32, tag="aB")
                    if j > 0:
                        nc.vector.tensor_tensor(
                            aB[0:na, :],
                            dA[0:na, j * C + 1:j * C + C + 1],
                            dA[0:na, j * C:j * C + C], mx)
                    else:
                        nc.vector.tensor_tensor(
                            aB[0:na, 0:1], dA[0:na, 1:2],
                            dA[0:na, RING * C:RING * C + 1], mx)
                        nc.vector.tensor_tensor(
                            aB[0:na, 1:C], dA[0:na, 2:C + 1],
                            dA[0:na, 1:C], mx)
                    nc.vector.tensor_tensor_scan(
                        DtB[s][0:na, xs:xs + C],
                        aB[0:na, :],
                        csB[s][0:na, xs:xs + C],
                        DtB[s][0:na, xs - 1:xs],
                        mx, sb)

                setup_strip(0)
                TOT = STAG * (S - 1) + NSTEP
                for T in range(TOT):
                    if T % STAG == 64 and T // STAG + 1 < S:
                        setup_strip(T // STAG + 1)
                    s_hi = min(T // STAG, S - 1)
                    s_lo = max(0, -(-(T - (NSTEP - 1)) // STAG))
                    for s in range(s_lo, s_hi + 1):
                        t = T - STAG * s
                        if 0 <= t < NSTEP:
                            emit_step(s, t)

                # similarity = 1/(1+distance); D' = -distance on partition 127
                r0 = pa.tile([1, 1], f32, tag="r0")
                nc.sync.dma_start(out=r0[:], in_=DtB[S - 1][P - 1:P, W2 - 1:W2])
                nc.vector.tensor_scalar(r0[:], r0[:], -1.0, 1.0, ml, ad)
                nc.vector.reciprocal(r0[:], r0[:])
                nc.sync.dma_start(out=out_ext[:], in_=r0[:])

    nc.finalize()
    return nc


_cache = {}


def _get_nc(N1, N2, D, C):
    key = (N1, N2, D, C)
    if key not in _cache:
        _cache[key] = _build(N1, N2, D, C)
    return _cache[key]


def run(trajectory1, trajectory2, C=32, trace=False):
    from concourse.bass_utils import run_bass_kernel_spmd

    N1, D = trajectory1.shape
    N2, _ = trajectory2.shape
    nc = _get_nc(N1, N2, D, C)
    t1 = np.ascontiguousarray(trajectory1, dtype=np.float32)
    t2 = np.ascontiguousarray(trajectory2, dtype=np.float32)
    # strip-cyclic host permutation: core r's slab blocks are original
    # 128-row blocks [r, 8+r, 16+r, 24+r] so AllGather #g lands original
    # rows [1024g, 1024(g+1)) in order.
    P = 128
    MT = N1 // P
    in_maps = []
    for r in range(NCORES):
        mine = list(range(r, MT, NCORES))
        rest = [b for b in range(MT) if b not in mine]
        idx = np.concatenate(
            [np.arange(b * P, (b + 1) * P) for b in mine + rest])
        in_maps.append({
            "trajectory1": np.ascontiguousarray(t1[idx]),
            "trajectory2": t2,
        })
    res = run_bass_kernel_spmd(
        nc, in_maps, list(range(NCORES)), trace=trace)
    out = res.results[0]["out"]
    return np.float32(out.reshape(())), res


def kernel(trajectory1, trajectory2):
    out, _ = run(trajectory1, trajectory2)
    return out


# revision 38
# speedup vs baseline: 2.6120x; 1.0143x over previous
"""DTW similarity kernel for Trainium2 (8 NeuronCores, SPMD bass/tile).

Per core (replicated; inputs identical on all cores):
  1. L2-normalize trajectory rows -> bf16 (DRAM bounce).
  2. DMA-transpose loads -> [D, N] bf16 operands in SBUF.
  3. cost = 1 - t1n @ t2n.T via PE matmuls -> bf16 cost matrix in DRAM.
  4. DTW DP over the NEGATED value space (D' = -D so min() becomes max(),
     letting max-pool do the up/diag candidate merge):
       - strips of 256 rows: partition p owns rows {2p, 2p+1}; skewed
         block wavefront, block width C, 2 rows chained per step.
       - cross-partition "up" for even rows via a PE shift matmul into
         PSUM (sh1: out[p]=in[p-1]; sh2 injects prev strip's last row).
       - aA = window-2 max-pool straight off PSUM; scanA computes the
         even row (tensor_tensor_scan, op0=max, op1=subtract).
       - odd row reads the fresh even row in SBUF (same partition):
         TT-max + scanB.
       - strips are emitted interleaved along the global wavefront so
         two strips pipeline across engines.
  5. similarity = 1/(1+distance) -> scalar output.
"""

import sys

sys.path.insert(0, "/opt/trn_rl_repo")

import numpy as np  # noqa: E402

BIG = 1e30
NCORES = 8


def _build(N1, N2, D, C):
    from concourse import bacc
    import concourse.bass as bass
    import concourse.mybir as mybir
    import concourse.tile as tile

    f32 = mybir.dt.float32
    bf16 = mybir.dt.bfloat16
    P = 128
    R = 2                      # rows per partition
    ROWS = P * R               # rows per strip
    S = N1 // ROWS             # strips
    B = N2 // C                # blocks per row
    NSTEP = B + P - 1          # wavefront steps per strip
    W2 = NSTEP * C + 1         # skewed D/cs tile width
    RING = 16                  # even-row D ring depth (blocks)
    RW = RING * C + 1
    STAG = P                   # wavefront stagger between strips
    KT = D // P
    NT = N2 // 512
    MT = N1 // P
    assert N1 % ROWS == 0 and N2 % C == 0 and D % P == 0 and N2 % 512 == 0

    MY = MT // NCORES          # row-blocks of t1 computed per core
    NROW = MY * P              # t1 rows used per core (host permutes them first)

    nc = bacc.Bacc(None, target_bir_lowering=False, debug=True, num_devices=NCORES)
    t1_ext = nc.dram_tensor("trajectory1", [N1, D], f32, kind="ExternalInput")
    t2_ext = nc.dram_tensor("trajectory2", [N2, D], f32, kind="ExternalInput")
    out_ext = nc.dram_tensor("out", [1, 1], f32, kind="ExternalOutput")
    # GEMM shard: AllGather #g over ranks assembles slab row-block g of every
    # core into agout rows [1024g, 1024(g+1)) = original rows (host
    # permutation is strip-cyclic: core r's g-th block is orig block 8g+r).

    ad = mybir.AluOpType.add
    ml = mybir.AluOpType.mult
    mx = mybir.AluOpType.max
    sb = mybir.AluOpType.subtract
    eq = mybir.AluOpType.is_equal
    AF = mybir.ActivationFunctionType
    PMAX = mybir.PoolFunctionType.max

    with tile.TileContext(nc) as tc:
        with tc.tile_pool(name="dram", bufs=1, space="DRAM") as dram:
            t1n_dram = dram.tile([NROW, D], bf16)
            t2n_dram = dram.tile([N2, D], bf16)
            slab = dram.tile([NROW, N2], bf16)
            agout = [dram.tile([NCORES * P, N2], bf16, addr_space="Shared",
                               name=f"agout{g}")
                     for g in range(MY)]

            # ---- phase A: normalize rows, write bf16 bounce ----
            with tc.tile_pool(name="norm", bufs=3) as pn, \
                 tc.tile_pool(name="norms", bufs=4) as psn:
                for src, dst, n in ((t1_ext, t1n_dram, NROW), (t2_ext, t2n_dram, N2)):
                    for i in range(n // P):
                        x = pn.tile([P, D], f32, tag="x")
                        nc.sync.dma_start(out=x[:], in_=src[i * P:(i + 1) * P, :])
                        sq = pn.tile([P, D], f32, tag="sq")
                        ss = psn.tile([P, 1], f32, tag="ss")
                        nc.scalar.activation(
                            out=sq[:], in_=x[:], func=AF.Square, accum_out=ss[:])
                        nc.scalar.activation(out=ss[:], in_=ss[:], func=AF.Sqrt)
                        nc.vector.tensor_scalar(ss[:], ss[:], 1e-8, None, ad)
                        r = psn.tile([P, 1], f32, tag="r")
                        nc.vector.reciprocal(r[:], ss[:])
                        y = pn.tile([P, D], bf16, tag="y")
                        nc.vector.tensor_tensor(
                            y[:], x[:], r[:].to_broadcast((P, D)), ml)
                        nc.sync.dma_start(out=dst[i * P:(i + 1) * P, :], in_=y[:])

            # ---- phase B+C: transposed loads + GEMM (cost in bf16) ----
            with tc.tile_pool(name="ops", bufs=1) as pg, \
                 tc.tile_pool(name="psum", bufs=4, space="PSUM") as pp, \
                 tc.tile_pool(name="bounce", bufs=4) as pb:
                t1T = []
                t2T = []
                for k in range(KT):
                    a = pg.tile([P, NROW], bf16, tag=f"t1T{k}")
                    nc.sync.dma_start_transpose(a[:], t1n_dram[:, k * P:(k + 1) * P])
                    t1T.append(a)
                for k in range(KT):
                    a = pg.tile([P, N2], bf16, tag=f"t2T{k}")
                    nc.sync.dma_start_transpose(a[:], t2n_dram[:, k * P:(k + 1) * P])
                    t2T.append(a)
                for m in range(MY):
                    for n in range(NT):
                        acc = pp.tile([P, 512], f32, tag="acc")
                        for k in range(KT):
                            nc.tensor.matmul(
                                acc[:],
                                t1T[k][:, m * P:(m + 1) * P],
                                t2T[k][:, n * 512:(n + 1) * 512],
                                start=(k == 0), stop=(k == KT - 1))
                        b = pb.tile([P, 512], bf16, tag="b")
                        nc.vector.tensor_scalar(b[:], acc[:], -1.0, 1.0, ml, ad)
                        nc.sync.dma_start(
                            out=slab[m * P:(m + 1) * P, n * 512:(n + 1) * 512],
                            in_=b[:])
                # pipelined all-gathers: group g = slab block g of every core
                for g in range(MY):
                    nc.gpsimd.collective_compute(
                        "AllGather",
                        mybir.AluOpType.bypass,
                        replica_groups=[list(range(NCORES))],
                        ins=[slab[g * P:(g + 1) * P, :].opt()],
                        outs=[agout[g][:, :].opt()],
                    )

            # ---- phase D: DTW DP (negated), interleaved wavefront ----
            with tc.tile_pool(name="dconst", bufs=1) as pc, \
                 tc.tile_pool(name="dcs", bufs=3) as pcs, \
                 tc.tile_pool(name="dDB", bufs=2) as pDB, \
                 tc.tile_pool(name="dDBh", bufs=3) as pDBh, \
                 tc.tile_pool(name="dDA", bufs=3) as pDA, \
                 tc.tile_pool(name="dps", bufs=8, space="PSUM") as pps, \
                 tc.tile_pool(name="da", bufs=8) as pa:
                # constants: shift matrices, -BIG tile, iotas
                iop = pc.tile([P, P], f32, tag="iop")   # value = partition idx
                iof = pc.tile([P, P], f32, tag="iof")   # value = free idx
                nc.gpsimd.iota(iof[:], [[1, P]], channel_multiplier=0,
                               allow_small_or_imprecise_dtypes=True)
                nc.gpsimd.iota(iop[:], [[0, P]], channel_multiplier=1,
                               allow_small_or_imprecise_dtypes=True)
                sh1 = pc.tile([P, P], f32, tag="sh1")   # sh1[k,m] = (k+1 == m)
                nc.vector.tensor_scalar(sh1[:], iop[:], 1.0, None, ad)
                nc.vector.tensor_tensor(sh1[:], sh1[:], iof[:], eq)
                sh2 = pc.tile([P, P], f32, tag="sh2")   # sh2[k,m] = (k==127)*(m==0)
                tmp = pc.tile([P, P], f32, tag="tmp")
                nc.vector.tensor_scalar(sh2[:], iop[:], float(P - 1), None, eq)
                nc.vector.tensor_scalar(tmp[:], iof[:], 0.0, None, eq)
                nc.vector.tensor_tensor(sh2[:], sh2[:], tmp[:], ml)
                sh2h = pc.tile([P, P], bf16, tag="sh2h")
                nc.vector.tensor_copy(sh2h[:], sh2[:])
                bigt = pc.tile([P, C + 1], bf16, tag="bigt")
                nc.vector.memset(bigt[:], -BIG)

                csA = {}
                csB = {}
                DtA = {}
                DtB = {}
                DtBh = {}

                def setup_strip(s):
                    # skewed bf16 cost loads: partition p <- rows 256s+2p / +1
                    g, srel = divmod(s * ROWS, NCORES * P)
                    for tag, par, dst in (("csA", 0, csA), ("csB", 1, csB)):
                        cs = pcs.tile([P, W2], bf16, tag=tag)
                        base = agout[g][:, :]
                        src = bass.AP(
                            tensor=base.tensor,
                            offset=base.offset + (srel + par) * N2,
                            ap=[[2 * N2 - C, P], [1, W2 - 1]])
                        nc.sync.dma_start(out=cs[:, 1:W2], in_=src)
                        dst[s] = cs
                    db = pDB.tile([P, W2], f32, tag="DtB")
                    nc.gpsimd.memset(db[:], -BIG)
                    DtB[s] = db
                    dbh = pDBh.tile([P, W2 - (P - 1) * C], bf16, tag="DtBh")
                    nc.gpsimd.memset(dbh[:], -BIG)
                    DtBh[s] = dbh
                    da = pDA.tile([P, RW], f32, tag="DtA")
                    nc.gpsimd.memset(da[:], -BIG)
                    DtA[s] = da

                def emit_step(s, t):
                    na = min(t, P - 1) + 1
                    tm = min(t, B - 1)
                    xs = t * C + 1
                    j = t % RING
                    ps = pps.tile([P, C + 1], f32, tag="ps")
                    # carry (prev strip's last row -> partition 0)
                    rhs2 = (bigt[:, 0:C + 1] if s == 0 else
                            DtBh[s - 1][:, tm * C:tm * C + C + 1])
                    nc.tensor.matmul(
                        ps[0:na, :], sh2h[:, 0:na], rhs2,
                        start=True, stop=(t == 0))
                    if t > 0:
                        nc.tensor.matmul(
                            ps[0:na, :], sh1[:, 0:na],
                            DtB[s][:, (t - 1) * C:t * C + 1],
                            start=False, stop=True)
                    # even row: copy shifted window off PSUM, then window-2 max
                    u = pa.tile([P, C + 1], f32, tag="u")
                    nc.scalar.activation(
                        out=u[0:na, :], in_=ps[0:na, :], func=AF.Copy)
                    aA = pa.tile([P, C], f32, tag="aA")
                    nc.vector.tensor_tensor(
                        aA[0:na, :], u[0:na, 1:C + 1], u[0:na, 0:C], mx)
                    if s == 0 and t == 0:
                        nc.vector.memset(aA[0:1, 0:1], 0.0)
                    dA = DtA[s]
                    pred = j * C if j > 0 else RING * C
                    nc.vector.tensor_tensor_scan(
                        dA[0:na, j * C + 1:j * C + C + 1],
                        aA[0:na, :],
                        csA[s][0:na, xs:xs + C],
                        dA[0:na, pred:pred + 1],
                        mx, sb)
                    # odd row: candidates from the fresh even row (same partition)
                    aB = pa.tile([P, C], f32, tag="aB")
                    if j > 0:
                        nc.vector.tensor_tensor(
                            aB[0:na, :],
                            dA[0:na, j * C + 1:j * C + C + 1],
                            dA[0:na, j * C:j * C + C], mx)
                    else:
                        nc.vector.tensor_tensor(
                            aB[0:na, 0:1], dA[0:na, 1:2],
                            dA[0:na, RING * C:RING * C + 1], mx)
                        nc.vector.tensor_tensor(
                            aB[0:na, 1:C], dA[0:na, 2:C + 1],
                            dA[0:na, 1:C], mx)
                    nc.vector.tensor_tensor_scan(
                        DtB[s][0:na, xs:xs + C],
                        aB[0:na, :],
                        csB[s][0:na, xs:xs + C],
                        DtB[s][0:na, xs - 1:xs],
                        mx, sb)
                    # bf16 mirror feeding the next strip's carry matmul (only
                    # partition 127 is consumed; full-partition copy keeps the
                    # activation at base partition 0, which the ISA requires)
                    if t >= P - 1 and s + 1 < S:
                        ho = xs - (P - 1) * C
                        nc.scalar.activation(
                            out=DtBh[s][0:P, ho:ho + C],
                            in_=DtB[s][0:P, xs:xs + C], func=AF.Copy)

                setup_strip(0)
                TOT = STAG * (S - 1) + NSTEP
                for T in range(TOT):
                    if T % STAG == 64 and T // STAG + 1 < S:
                        setup_strip(T // STAG + 1)
                    s_hi = min(T // STAG, S - 1)
                    s_lo = max(0, -(-(T - (NSTEP - 1)) // STAG))
                    for s in range(s_lo, s_hi + 1):
                        t = T - STAG * s
                        if 0 <= t < NSTEP:
                            emit_step(s, t)

                # similarity = 1/(1+distance); D' = -distance on partition 127
                r0 = pa.tile([1, 1], f32, tag="r0")
                nc.sync.dma_start(out=r0[:], in_=DtB[S - 1][P - 1:P, W2 - 1:W2])
                nc.vector.tensor_scalar(r0[:], r0[:], -1.0, 1.0, ml, ad)
                nc.vector.reciprocal(r0[:], r0[:])
                nc.sync.dma_start(out=out_ext[:], in_=r0[:])

    nc.finalize()
    return nc


_cache = {}


def _get_nc(N1, N2, D, C):
    key = (N1, N2, D, C)
    if key not in _cache:
        _cache[key] = _build(N1, N2, D, C)
    return _cache[key]


def run(trajectory1, trajectory2, C=32, trace=False):
    from concourse.bass_utils import run_bass_kernel_spmd

    N1, D = trajectory1.shape
    N2, _ = trajectory2.shape
    nc = _get_nc(N1, N2, D, C)
    t1 = np.ascontiguousarray(trajectory1, dtype=np.float32)
    t2 = np.ascontiguousarray(trajectory2, dtype=np.float32)
    # strip-cyclic host permutation: core r's slab blocks are original
    # 128-row blocks [r, 8+r, 16+r, 24+r] so AllGather #g lands original
    # rows [1024g, 1024(g+1)) in order.
    P = 128
    MT = N1 // P
    in_maps = []
    for r in range(NCORES):
        mine = list(range(r, MT, NCORES))
        rest = [b for b in range(MT) if b not in mine]
        idx = np.concatenate(
            [np.arange(b * P, (b + 1) * P) for b in mine + rest])
        in_maps.append({
            "trajectory1": np.ascontiguousarray(t1[idx]),
            "trajectory2": t2,
        })
    res = run_bass_kernel_spmd(
        nc, in_maps, list(range(NCORES)), trace=trace)
    out = res.results[0]["out"]
    return np.float32(out.reshape(())), res


def kernel(trajectory1, trajectory2):
    out, _ = run(trajectory1, trajectory2)
    return out
